# revision 76
# baseline (speedup 1.0000x reference)
"""Trainium2 Bass kernel for multi-head attention graph scatter.

Computes, for each of 8 heads h (one NeuronCore per head):
    q_h = query @ w_q[:, h*32:(h+1)*32]          # [3000, 32]
    k_h = key_emb @ w_k[:, h*32:(h+1)*32]        # [4096, 32]
    attn_h = softmax(q_h @ k_h.T / sqrt(32))     # [3000, 4096]
    graphs[h, qt, :] = attn_h                    # [4096, 4096], rest zeros

kernel(**inputs) takes the full (unsharded) numpy inputs and returns the
full [8, 4096, 4096] float32 output.

Schedule notes: the DMA device serializes all HBM traffic at 360 GB/s,
so the floor is reads (20.6us) + writes (136.5us) plus startup and tail
latency.  All loads and stores ride the sync queue in an explicitly
interleaved order (keys first, deferred query groups as gap fillers
between the first writes).  The startup key-group pipeline
(load -> PE transpose -> PSUM drain -> projection -> scores -> exp) is
balanced so DVE and Act each stay under the 1,456ns/group load pace;
only DVE/Act may drain PSUM (GPSIMD cannot touch PSUM on real HW).
Tile 0's softmax is chunked (512-wide last exp, per-slice normalize and
write) so the first store issues as early as possible.
"""

import math
import sys

import numpy as np

if "/opt/trn_rl_repo" not in sys.path:
    sys.path.insert(0, "/opt/trn_rl_repo")

N_HEAD = 8
D_K = 32
CONCEPT_NUM = 4096
MASK_NUM = 3000
INPUT_DIM = 256

P = 128  # SBUF partitions
NBLK = 512  # matmul moving-dim tile (one PSUM bank of f32)

_BUILD_CACHE = {}


def _build_module():
    """Build the per-core Bass module (identical on all 8 cores; inputs differ)."""
    import concourse.bacc as bacc
    import concourse.mybir as mybir
    import concourse.tile as tile
    from concourse.masks import make_identity

    f32 = mybir.dt.float32
    f32r = mybir.dt.float32r
    SCALE = 1.0 / math.sqrt(D_K)

    nc = bacc.Bacc("TRN2", target_bir_lowering=False, debug=False, num_devices=N_HEAD)

    # f32r == f32 bits; declaring the inputs f32r lets the load tiles be f32r
    # (cheaper PE transposes) without a casting DMA
    query = nc.dram_tensor("query", [MASK_NUM, INPUT_DIM], f32r, kind="ExternalInput")
    key_emb = nc.dram_tensor("key_emb", [CONCEPT_NUM, INPUT_DIM], f32r, kind="ExternalInput")
    w_qh = nc.dram_tensor("w_qh", [INPUT_DIM, D_K], f32, kind="ExternalInput")
    w_kh = nc.dram_tensor("w_kh", [INPUT_DIM, D_K], f32, kind="ExternalInput")
    graphs = nc.dram_tensor("graphs", [CONCEPT_NUM, CONCEPT_NUM], f32, kind="ExternalOutput")

    # mask-dim tiling: 3000 = 23*128 + 56
    m_tiles = [P] * (MASK_NUM // P) + ([MASK_NUM % P] if MASK_NUM % P else [])
    n_mt = len(m_tiles)
    n_kc = CONCEPT_NUM // NBLK  # 8 concept chunks of 512
    q_chunks = [NBLK] * (MASK_NUM // NBLK) + ([MASK_NUM % NBLK] if MASK_NUM % NBLK else [])
    n_qc = len(q_chunks)  # 6 mask chunks (5x512 + 440)
    n_qt_full = MASK_NUM // P  # 23 full query row-tiles
    mrem = MASK_NUM - n_qt_full * P  # 56

    with tile.TileContext(nc) as tc:
        with (
            tc.tile_pool(name="const", bufs=1) as const_pool,
            tc.tile_pool(name="loads", bufs=6) as loads,
            tc.tile_pool(name="qstash", bufs=5) as qstash,
            tc.tile_pool(name="trans", bufs=1) as trans_pool,
            tc.tile_pool(name="proj", bufs=1) as proj_pool,
            tc.tile_pool(name="stats", bufs=4) as stats,
            tc.tile_pool(name="expp", bufs=4) as expp,
            tc.tile_pool(name="tpsum", bufs=3, space="PSUM") as tpsum,
            tc.tile_pool(name="ppsum", bufs=1, space="PSUM") as ppsum,
            tc.tile_pool(name="mpsum", bufs=2, space="PSUM") as mpsum,
        ):
            # f32r identity: transposes are modeled off the moving operand's
            # dtype (1.5 cyc/row for f32r vs 2.0 for f32)
            identity_f32 = const_pool.tile([P, P], f32)
            make_identity(nc, identity_f32)
            identity = const_pool.tile([P, P], f32r)
            nc.vector.tensor_copy(identity[:], identity_f32[:])

            def warm_pe(n):
                """Dummy transposes to start/hold the PE p-state ramp."""
                for _ in range(n):
                    wtp = tpsum.tile([P, 2 * P], f32r, tag="tp", name="wtp")
                    nc.tensor.transpose(wtp[:, :P], identity[:], identity[:])

            # w slices in lhsT layout: [128, 2, 32] where [p, a, j] = w[a*128+p, j];
            # rounded to f32r for the (f32r) projection matmuls.
            wq_f32 = const_pool.tile([P, 2, D_K], f32)
            wk_f32 = const_pool.tile([P, 2, D_K], f32)
            wq_sb = const_pool.tile([P, 2, D_K], f32r)
            wk_sb = const_pool.tile([P, 2, D_K], f32r)

            def emit_w_loads():
                # scalar (Activation) queue: keeps the tiny w transfers off the
                # sync queue so the q/k load stream has no decode bubbles.
                # Casts run on the otherwise-idle Pool engine — a cast waiting
                # on the w loads must not block the DVE/Act copy queues.
                nc.scalar.dma_start(wq_f32[:], w_qh.ap().rearrange("(a p) j -> p a j", p=P))
                nc.scalar.dma_start(wk_f32[:], w_kh.ap().rearrange("(a p) j -> p a j", p=P))
                nc.gpsimd.tensor_copy(wq_sb[:], wq_f32[:])
                nc.gpsimd.tensor_copy(wk_sb[:], wk_f32[:])

            # transposed input staging (f32r, rounded by the PSUM->SBUF copies)
            keyT = [
                [trans_pool.tile([P, NBLK], f32r, tag=f"keyT{a}_{j}", name=f"keyT{a}_{j}") for j in range(n_kc)]
                for a in range(2)
            ]
            queryT = [
                [trans_pool.tile([P, q_chunks[j]], f32r, tag=f"queryT{a}_{j}", name=f"queryT{a}_{j}") for j in range(n_qc)]
                for a in range(2)
            ]
            kT = [proj_pool.tile([D_K, NBLK], f32r, tag=f"kT_{j}", name=f"kT_{j}") for j in range(n_kc)]
            qT = [proj_pool.tile([D_K, q_chunks[j]], f32r, tag=f"qT_{j}", name=f"qT_{j}") for j in range(n_qc)]

            # ---------- helpers ----------
            copy_flip = [0]

            def transpose_quad(srcs, dst, col, eng=None):
                """PE-transpose up to four [rows<=128, 128] blocks into one
                [128, 512] PSUM tile, then ONE wide copy into dst[:, col:...].
                Halves the tpsum-ring turnover and the copy count vs
                per-pair staging — the ring round-trip was the startup
                pipeline's latency limiter."""
                tp = tpsum.tile([P, 4 * P], f32r, tag="tp", name="tp")
                w = 0
                for src in srcs:
                    if src is None:
                        continue
                    r = src.shape[0]
                    nc.tensor.transpose(tp[:, w : w + r], src, identity[:r, :r])
                    w += r
                # PSUM can only be drained by DVE/Act (GPSIMD may not touch
                # PSUM on real HW).  Alternate the two per-group copies so
                # each engine stays under the 1,456ns/group load pace; the
                # caller can force an engine (last key groups -> DVE so the
                # critical tail never queues behind an Act exp).
                if eng is None:
                    copy_flip[0] = (copy_flip[0] + 1) % 2
                    eng = nc.vector if copy_flip[0] == 1 else None
                if eng is nc.vector:
                    nc.vector.tensor_copy(dst[:, col : col + w], tp[:, :w])
                else:
                    nc.scalar.copy(dst[:, col : col + w], tp[:, :w])

            def load_query_tiles(g):
                """DMA query row-tiles 4g..4g+3 (or the 440+56 tail) on sync.

                Deferred groups get their own pool so these loads never wait
                on key-tile transpose progress for a buffer."""
                if g == 0:
                    qtile = loads.tile([P, 4, INPUT_DIM], f32r, tag="ld", name="qload")
                else:
                    qtile = qstash.tile([P, 4, INPUT_DIM], f32r, tag="qs", name="qstash")
                t0 = g * 4
                t1 = min(t0 + 4, n_qt_full)
                if t1 > t0:
                    src = query.ap()[t0 * P : t1 * P, :].rearrange("(t p) d -> p t d", p=P)
                    nc.sync.dma_start(qtile[:, : t1 - t0, :], src)
                if g == 5:
                    nc.sync.dma_start(qtile[:mrem, 3, :], query.ap()[n_qt_full * P :, :])
                return qtile

            def transpose_query_group(g, qtile):
                t0 = g * 4
                t1 = min(t0 + 4, n_qt_full)
                has_tail = g == 5
                n_full = t1 - t0
                for a in range(2):
                    srcs = [qtile[:, t, a * P : (a + 1) * P] for t in range(n_full)]
                    if has_tail:
                        srcs.append(qtile[:mrem, 3, a * P : (a + 1) * P])
                    transpose_quad(srcs, queryT[a][g], 0)

            def project(dst, srcT, w_sb, width, copy_eng=None):
                ps = ppsum.tile([D_K, NBLK], f32, tag="pps", name="pps")
                nc.tensor.matmul(ps[:, :width], w_sb[:, 0, :], srcT[0][:], start=True, stop=False)
                nc.tensor.matmul(ps[:, :width], w_sb[:, 1, :], srcT[1][:], start=False, stop=True)
                # DVE by default (GPSIMD may not read PSUM on real HW)
                if copy_eng is nc.scalar:
                    nc.scalar.copy(dst[:], ps[:, :width])
                else:
                    nc.vector.tensor_copy(dst[:], ps[:, :width])

            def finish_query_chunk(g, qtile):
                transpose_query_group(g, qtile)
                project(qT[g], [queryT[0][g], queryT[1][g]], wq_sb, q_chunks[g])

            def scores_chunk(i, ps, j2_off, j):
                """One [mt, 512] scores matmul for m-tile i into psum slice j2_off."""
                mt = m_tiles[i]
                cj = i // 4
                c0 = i * P - cj * NBLK
                nc.tensor.matmul(
                    ps[:mt, j2_off * NBLK : (j2_off + 1) * NBLK],
                    qT[cj][:, c0 : c0 + mt],
                    kT[j][:],
                    start=True,
                    stop=True,
                )

            def exp_chunk(i, ps, exp_dst, sums, h4):
                mt = m_tiles[i]
                nc.scalar.activation(
                    exp_dst[:mt, h4 * 2 * NBLK : (h4 + 1) * 2 * NBLK],
                    ps[:mt, :],
                    mybir.ActivationFunctionType.Exp,
                    scale=SCALE,
                    accum_out=sums[:mt, h4 : h4 + 1],
                )

            def normalize(i, exp_dst, sums, tag, ncols=4):
                mt = m_tiles[i]
                tot = stats.tile([P, 1], f32, tag=f"tot{tag}", name="tot")
                rec = stats.tile([P, 1], f32, tag=f"rec{tag}", name="rec")
                nc.vector.tensor_reduce(
                    tot[:mt], sums[:mt, :ncols], axis=mybir.AxisListType.X, op=mybir.AluOpType.add
                )
                nc.vector.reciprocal(rec[:mt], tot[:mt])
                nc.vector.tensor_scalar_mul(exp_dst[:mt, :], exp_dst[:mt, :], rec[:mt])

            def softmax_tile(i, exp_dst, fine=False):
                """Full scores+exp+normalize for m-tile i into exp_dst [P, C].

                fine=True uses 512-wide exps (8 instead of 4): slightly more
                Act time but shorter Act slices, so an exp on tile-0's critical
                chain is blocked for less when this tile runs concurrently.
                """
                mt = m_tiles[i]
                sums = stats.tile([P, n_kc], f32, tag="sums", name="sums")
                ncols = n_kc if fine else 4
                for h4 in range(4):
                    ps = mpsum.tile([P, 2 * NBLK], f32, tag="mps", name="mps")
                    for j2 in range(2):
                        scores_chunk(i, ps, j2, h4 * 2 + j2)
                        if fine:
                            nc.scalar.activation(
                                exp_dst[:mt, (h4 * 2 + j2) * NBLK : (h4 * 2 + j2 + 1) * NBLK],
                                ps[:mt, j2 * NBLK : (j2 + 1) * NBLK],
                                mybir.ActivationFunctionType.Exp,
                                scale=SCALE,
                                accum_out=sums[:mt, h4 * 2 + j2 : h4 * 2 + j2 + 1],
                            )
                    if not fine:
                        exp_chunk(i, ps, exp_dst, sums, h4)
                normalize(i, exp_dst, sums, "", ncols)

            def out_dma(i, exp_dst):
                # all writes on sync: HWDGE latency (~1.3us) beats SWDGE
                # (~1.7us+gen), and a single queue gives explicit control of
                # the DMA device's service order (writes vs deferred loads)
                mt = m_tiles[i]
                nc.sync.dma_start(graphs.ap()[i * P : i * P + mt, :], exp_dst[:mt, :])

            # ---------- startup ----------
            # sync-queue load order: query g0, key groups 0..7, then the
            # deferred query groups 1..5 (they fill the DMA device while
            # tile-0's first writes become ready).  w rides the scalar queue.
            # PE: warmup until g0 lands (+900ns sem prop), then g0 transposes
            # + qT0 project, then per key group transposes+project+tile-0
            # scores, paced by the loads (fillers hold the p-state ramp).
            emit_w_loads()
            warm_pe(14)
            qtile0 = load_query_tiles(0)
            transpose_query_group(0, qtile0)
            project(qT[0], [queryT[0][0], queryT[1][0]], wq_sb, q_chunks[0])

            key_r = key_emb.ap().rearrange("(t p) d -> p t d", p=P)  # [128, 32, 256]
            exp0 = expp.tile([P, CONCEPT_NUM], f32, tag="exp", name="exp0")
            # per-512 exp chunks for tile 0 (8 partial sums): the last exp in
            # tile-0's critical chain is then only 512 wide
            sums0 = stats.tile([P, n_kc], f32, tag="sums", name="sums0")
            ps0 = None
            for j in range(n_kc):  # 8 key groups of 4 row-tiles (0.5 MB loads)
                ktile = loads.tile([P, 4, INPUT_DIM], f32r, tag="ld", name="kload")
                nc.sync.dma_start(ktile[:], key_r[:, j * 4 : (j + 1) * 4, :])
                for a in range(2):
                    transpose_quad(
                        [ktile[:, t, a * P : (a + 1) * P] for t in range(4)],
                        keyT[a][j],
                        0,
                        eng=nc.vector if j == n_kc - 1 else None,
                    )
                project(kT[j], [keyT[0][j], keyT[1][j]], wk_sb, NBLK)
                if j % 2 == 0:
                    ps0 = mpsum.tile([P, 2 * NBLK], f32, tag="mps", name="mps")
                scores_chunk(0, ps0, j % 2, j)
                # coarse 1024-wide exps for chunk pairs (0,1),(2,3),(4,5)
                # (less Act time), fine 512-wide for chunks 6 and 7 (shorter
                # critical tail).  Partial sums land in sums0 cols 0..4
                # COMPACTLY — the reduce must see only written columns.
                if j >= 6:
                    nc.scalar.activation(
                        exp0[:, j * NBLK : (j + 1) * NBLK],
                        ps0[:, (j % 2) * NBLK : (j % 2 + 1) * NBLK],
                        mybir.ActivationFunctionType.Exp,
                        scale=SCALE,
                        accum_out=sums0[:, j - 3 : j - 2],
                    )
                elif j % 2 == 1:
                    nc.scalar.activation(
                        exp0[:, (j - 1) * NBLK : (j + 1) * NBLK],
                        ps0[:, :],
                        mybir.ActivationFunctionType.Exp,
                        scale=SCALE,
                        accum_out=sums0[:, j // 2 : j // 2 + 1],
                    )

            # deferred query loads g1..g4 on sync: transfers run back-to-back
            # behind the key stream, keeping the DMA device busy while tile-0's
            # normalize finishes (g5 is emitted later, between the first
            # writes, as the gap filler for tile-1/2 readiness jitter).
            # Transposes/projections happen in the main loop.
            q_stash = {g: load_query_tiles(g) for g in range(1, n_qc - 1)}

            # tile 0: normalize + write in quarters so the first HBM write
            # starts as soon as possible (startup latency is the critical path)
            tot0 = stats.tile([P, 1], f32, tag="tot_t0", name="tot0")
            part0 = stats.tile([P, 1], f32, tag="part_t0", name="part0")
            rec0 = stats.tile([P, 1], f32, tag="rec_t0", name="rec0")
            # pre-sum the four early partials (ready well before exp j7) so
            # the final-exp -> reciprocal chain is one [P,1] add shorter
            nc.vector.tensor_reduce(
                part0[:], sums0[:, :4], axis=mybir.AxisListType.X, op=mybir.AluOpType.add
            )
            nc.vector.tensor_tensor(
                tot0[:], part0[:], sums0[:, 4:5], op=mybir.AluOpType.add
            )
            nc.vector.reciprocal(rec0[:], tot0[:])
            # first slice small (512) so the first write issues ASAP
            bounds = [0, 512, 1536, 2560, 3584, CONCEPT_NUM]
            for qi in range(len(bounds) - 1):
                sl = slice(bounds[qi], bounds[qi + 1])
                nc.vector.tensor_scalar_mul(exp0[:, sl], exp0[:, sl], rec0[:])
                nc.sync.dma_start(graphs.ap()[0:P, sl], exp0[:, sl])


            # ---------- main loop; query chunks transposed one chunk ahead ----------
            done_qc = 1
            for i in range(1, n_mt):
                # transpose+project query chunk g one tile before it is needed;
                # the wait hint keeps this slack work (PE transposes + DVE/Pool
                # copies) out of tile-0's critical startup window
                if i % 4 == 3 and done_qc < n_qc and done_qc == (i + 1) // 4:
                    with tc.tile_wait_until(0.022 + 0.004 * (done_qc - 1)):
                        finish_query_chunk(done_qc, q_stash[done_qc])
                    done_qc += 1
                exp_t = expp.tile([P, CONCEPT_NUM], f32, tag="exp", name="exp_t")
                softmax_tile(i, exp_t)
                out_dma(i, exp_t)
                if i == 1:
                    # final deferred load: fills the DMA stream between the
                    # first writes while tiles 2+ catch up
                    q_stash[n_qc - 1] = load_query_tiles(n_qc - 1)
            while done_qc < n_qc:  # safety (should not trigger)
                finish_query_chunk(done_qc, q_stash[done_qc])
                done_qc += 1

    nc.compile()
    return nc


def _get_module():
    if "nc" not in _BUILD_CACHE:
        _BUILD_CACHE["nc"] = _build_module()
    return _BUILD_CACHE["nc"]


def kernel(qt, query, key_emb, w_q, w_k):
    from concourse.bass_utils import run_bass_kernel_spmd

    qt = np.asarray(qt)
    query = np.ascontiguousarray(np.asarray(query, dtype=np.float32))
    key_emb = np.ascontiguousarray(np.asarray(key_emb, dtype=np.float32))
    w_q = np.asarray(w_q, dtype=np.float32)
    w_k = np.asarray(w_k, dtype=np.float32)

    nc = _get_module()
    in_maps = []
    for h in range(N_HEAD):
        in_maps.append(
            {
                "query": query,
                "key_emb": key_emb,
                "w_qh": np.ascontiguousarray(w_q[:, h * D_K : (h + 1) * D_K]),
                "w_kh": np.ascontiguousarray(w_k[:, h * D_K : (h + 1) * D_K]),
            }
        )
    res = run_bass_kernel_spmd(nc, in_maps, core_ids=list(range(N_HEAD)))
    out = np.stack([res.results[h]["graphs"] for h in range(N_HEAD)], axis=0)

    # Device assumes qt == arange(3000) (rows land at graph rows 0..2999,
    # remaining rows stay zero). Remap on host for any other qt.
    if not np.array_equal(qt, np.arange(MASK_NUM)):
        full = np.zeros((N_HEAD, CONCEPT_NUM, CONCEPT_NUM), dtype=np.float32)
        full[:, qt.astype(np.int64), :] = out[:, :MASK_NUM, :]
        out = full
    return out
